# revision 58
# baseline (speedup 1.0000x reference)
# Distributed Trainium2 kernel for the GQA attention block
# (nn_Attention_52621939311076).
#
# Sharding: tensor-parallel over heads across 8 NeuronCores. Core c owns
# q-heads [8c, 8c+8) and kv-head c (GQA group stays local). x is replicated,
# wq/wk/wv are sharded on the output dim, wo on the input dim; partial wo
# outputs are summed with an on-device ReduceScatter and the rank slices are
# concatenated on the host.
#
# Everything on device lives in a transposed [feature, seq] layout so that no
# on-chip transposes are needed anywhere:
#   - projections produce Q^T/K^T (head_dim on partitions) and V in [s, d],
#   - RMSNorm reduction over head_dim uses a ones-matmul (partition reduce),
#   - RoPE pairs are (even, odd) partition halves via a host-side permutation
#     of the wq/wk output dims,
#   - attention computes S^T = K^T.T-stationary @ Q^T, softmax row sums via a
#     ones-matmul, O^T = V-stationary @ P^T,
#   - the wo matmul consumes O^T directly.
# Matmuls run in bf16 (4x the fp32 TensorE rate), accumulating in fp32 PSUM.
import numpy as np
import ml_dtypes

import concourse.bass as bass
import concourse.bacc as bacc
import concourse.mybir as mybir
import concourse.tile as tile
from concourse.bass_utils import run_bass_kernel_spmd

# enable the neuronxcc ldweights optimization (measured ~12us faster)
try:
    from concourse import compiler_utils as _cu
    _fl = _cu.get_compiler_flags()
    _cu.set_compiler_flags([f.replace("--enable-ldw-opt=false",
                                      "--enable-ldw-opt=true") for f in _fl])
except Exception:
    pass

BF16 = mybir.dt.bfloat16
F32 = mybir.dt.float32
FP8 = mybir.dt.float8e4
NPBF16 = ml_dtypes.bfloat16

N_CORES = 8
S = 2048          # sequence length
D = 5120          # model dim
H = 64            # q heads (global)
KVH = 8           # kv heads (global)
HD = 128          # head dim
HQ = H // N_CORES  # q heads per core
DC = D // 128     # contraction chunks for the projections
SB = S // 128     # 128-row seq blocks
NG = S // 512     # 512-col seq groups
DG = D // 512     # 512-col output groups for wo
EPS = 1e-6

_cache = {}
RS_CHUNKED = True



def _build(causal: bool):
    nc = bacc.Bacc("TRN2", target_bir_lowering=False, debug=False,
                   num_devices=N_CORES)

    xt_e = nc.dram_tensor("xt", [NG, 128, DC, 512], BF16, kind="ExternalInput")
    wq_e = nc.dram_tensor("wq", [HQ, 128, DC, 128], BF16, kind="ExternalInput")
    wk_e = nc.dram_tensor("wk", [128, DC, 128], BF16, kind="ExternalInput")
    wv_e = nc.dram_tensor("wv", [128, DC, 128], BF16, kind="ExternalInput")
    wo_e = nc.dram_tensor("wo", [HQ, 128, DG, 512], BF16, kind="ExternalInput")
    cos_e = nc.dram_tensor("cos", [128, S], BF16, kind="ExternalInput")
    sin_e = nc.dram_tensor("sin", [128, S], BF16, kind="ExternalInput")
    swp_e = nc.dram_tensor("swp", [128, 128], BF16, kind="ExternalInput")
    qw_e = nc.dram_tensor("qw", [128, 1], F32, kind="ExternalInput")
    kw_e = nc.dram_tensor("kw", [128, 1], F32, kind="ExternalInput")
    idt_e = nc.dram_tensor("idt", [128, 128], BF16, kind="ExternalInput")
    if causal:
        mask_e = nc.dram_tensor("mask", [SB, 128, 128], BF16, kind="ExternalInput")
    else:
        mask_e = nc.dram_tensor("mask", [SB, NG, 128, 512], F32,
                                kind="ExternalInput")
    out_e = nc.dram_tensor("out", [S // N_CORES, D], BF16, kind="ExternalOutput")

    mult = mybir.AluOpType.mult
    Exp = mybir.ActivationFunctionType.Exp
    Sqrt = mybir.ActivationFunctionType.Sqrt
    Square = mybir.ActivationFunctionType.Square

    with tile.TileContext(nc) as tc, \
         tc.tile_pool(name="persist", bufs=1) as persist:
        def single(shape, dtype, name):
            return persist.tile(shape, dtype, name=name, tag=name)

        # ---- persistent SBUF tensors -------------------------------------
        QR = single([128, HQ * S], BF16, "QR")     # roped q, [d, s] per head
        KR = single([128, S], BF16, "KR")          # roped k, [d, s]
        Vsd = single([128, S], BF16, "Vsd")        # v in [s, d], s-block b at cols b*128
        cosT = single([128, S], BF16, "cosT")   # cos duplicated on both halves
        sinT = single([128, S], BF16, "sinT")   # [-sin; +sin]
        swp_t = single([128, 128], BF16, "swp_t")
        qw_t = single([128, 1], F32, "qw_t")
        kw_t = single([128, 1], F32, "kw_t")
        ones_f = single([128, 128], BF16, "ones_f")  # full ones: bcast rowsum
        ones8 = single([128, 256], FP8, "ones8")  # fp8 pair-ones: DR rowsum
        eps_t = single([128, 1], F32, "eps_t")
        ident = single([128, 128], BF16, "ident")
        if causal:
            maskT = single([128, SB * 128], BF16, "maskT")

        nc.gpsimd.dma_start(out=cosT[:, :], in_=cos_e[:, :])
        nc.gpsimd.dma_start(out=sinT[:, :], in_=sin_e[:, :])
        nc.gpsimd.dma_start(out=swp_t[:, :], in_=swp_e[:, :])
        nc.gpsimd.dma_start(out=qw_t[:, :], in_=qw_e[:, :])
        nc.gpsimd.dma_start(out=kw_t[:, :], in_=kw_e[:, :])
        nc.vector.memset(ones_f[:, :], 1.0)
        nc.vector.memset(ones8[:, :], 1.0)
        nc.vector.memset(eps_t[:, :], EPS)
        nc.gpsimd.dma_start(out=ident[:, :], in_=idt_e[:, :])
        if causal:
            for b in range(SB):
                nc.gpsimd.dma_start(out=maskT[:, b * 128:(b + 1) * 128],
                                  in_=mask_e[b])

        # ---- stage 1+2: projections + rmsnorm + rope ---------------------
        def norm_rope(pj, w_ap, dst, dst_cols, sg, sqp, bcp, swsb, stats,
                      tmps):
            """pj: PSUM [128,512] projection block; writes roped dst[:, dst_cols]."""
            sq = sqp.tile([128, 512], BF16, tag="sq")
            nc.scalar.activation(sq[:, :], pj[:, :], Square)
            # partition-sum of sq broadcast to all 128 partitions in one matmul
            bc = bcp.tile([128, 512], F32, tag="bc")
            nc.tensor.matmul(bc[:, :], ones_f[:, :], sq[:, :], start=True, stop=True)
            rstd = stats.tile([128, 512], F32, tag="rstd")
            nc.scalar.activation(rstd[:, :], bc[:, :], Sqrt, bias=eps_t[:, :],
                                 scale=1.0 / HD)
            rec = stats.tile([128, 512], F32, tag="rec")
            nc.vector.reciprocal_approx_fast(rec[:, :], rstd[:, :])
            qn = stats.tile([128, 512], BF16, tag="qn")
            # qn = (pj * w) * rec  -- normalized, weighted, cast to bf16
            nc.vector.scalar_tensor_tensor(qn[:, :], pj[:, :], w_ap, rec[:, :],
                                           op0=mult, op1=mult)
            # rope: out = qn*cos2 + swap_halves(qn)*[-sin; sin]; the partition
            # half-swap runs on the DMA engines, not the PE.
            cs = cosT[:, sg * 512:(sg + 1) * 512]
            sn = sinT[:, sg * 512:(sg + 1) * 512]
            sw = swsb.tile([128, 512], BF16, tag="sw")
            nc.sync.dma_start(out=sw[0:64, :], in_=qn[64:128, :])
            nc.sync.dma_start(out=sw[64:128, :], in_=qn[0:64, :])
            t1 = tmps.tile([128, 512], BF16, tag="t1")
            t2 = tmps.tile([128, 512], BF16, tag="t2")
            nc.vector.tensor_mul(t1[:, :], qn[:, :], cs)
            nc.vector.tensor_mul(t2[:, :], sw[:, :], sn)
            nc.vector.tensor_add(dst[:, dst_cols], t1[:, :], t2[:, :])

        with tc.tile_pool(name="xp", bufs=3) as xp, \
             tc.tile_pool(name="wqp", bufs=2) as wqp, \
             tc.tile_pool(name="wkvp", bufs=2) as wkvp, \
             tc.tile_pool(name="sqp", bufs=2) as sqp, \
             tc.tile_pool(name="stats", bufs=3) as stats, \
             tc.tile_pool(name="tmps", bufs=4) as tmps, \
             tc.tile_pool(name="vtp", bufs=2) as vtp, \
             tc.tile_pool(name="swsb", bufs=3) as swsb, \
             tc.tile_pool(name="pj", bufs=3, space="PSUM") as pjp, \
             tc.tile_pool(name="bcp", bufs=2, space="PSUM") as bcp, \
             tc.tile_pool(name="pv", bufs=1, space="PSUM") as pvp, \
             tc.tile_pool(name="trp", bufs=2, space="PSUM") as trp:
            for sg in range(NG):
                # x^T for this s-group: two half tiles [128, 20*512]
                xhs = []
                for hf in range(2):
                    xh = xp.tile([128, 20 * 512], BF16, tag="xh")
                    nc.sync.dma_start(
                        out=xh[:, :].rearrange("p (a m) -> p a m", a=20),
                        in_=xt_e[sg, :, hf * 20:(hf + 1) * 20, :])
                    xhs.append(xh)

                def xs(dc, c0, w):
                    return xhs[dc // 20][:, (dc % 20) * 512 + c0:
                                         (dc % 20) * 512 + c0 + w]

                # K/V weights for this s-group (reloaded each sg; cheap)
                wk_t = wkvp.tile([128, DC * 128], BF16, tag="wkv")
                nc.sync.dma_start(
                    out=wk_t[:, :].rearrange("p (a m) -> p a m", a=DC),
                    in_=wk_e[:, :, :])
                wv_t = wkvp.tile([128, DC * 128], BF16, tag="wkv")
                nc.sync.dma_start(
                    out=wv_t[:, :].rearrange("p (a m) -> p a m", a=DC),
                    in_=wv_e[:, :, :])
                cols = slice(sg * 512, (sg + 1) * 512)
                pk = pjp.tile([128, 512], F32, tag="pj")
                for dc in range(DC):
                    nc.tensor.matmul(pk[:, :],
                                     wk_t[:, dc * 128:(dc + 1) * 128],
                                     xs(dc, 0, 512),
                                     start=(dc == 0), stop=(dc == DC - 1))
                norm_rope(pk, kw_t[:, :], KR, cols, sg, sqp, bcp, swsb,
                          stats, tmps)
                # V^T [d, s] with 512-wide matmuls (like K), then PE-transpose
                # each 128x128 block into the [s, d] layout the AV matmul needs.
                pvt = pvp.tile([128, 512], F32, tag="pv")
                for dc in range(DC):
                    nc.tensor.matmul(pvt[:, :],
                                     wv_t[:, dc * 128:(dc + 1) * 128],
                                     xs(dc, 0, 512),
                                     start=(dc == 0), stop=(dc == DC - 1))
                vt_sb = vtp.tile([128, 512], BF16, tag="vt")
                nc.vector.tensor_copy(vt_sb[:, :], pvt[:, :])
                for sb4 in range(4):
                    sb = sg * 4 + sb4
                    tr = trp.tile([128, 128], BF16, tag="tr")
                    nc.tensor.transpose(tr[:, :],
                                        vt_sb[:, sb4 * 128:(sb4 + 1) * 128],
                                        ident[:, :])
                    nc.vector.tensor_copy(Vsd[:, sb * 128:(sb + 1) * 128],
                                          tr[:, :])
                # Q heads
                for qb in range(HQ):
                    wq_t = wqp.tile([128, DC * 128], BF16, tag="wq")
                    nc.sync.dma_start(
                        out=wq_t[:, :].rearrange("p (a m) -> p a m", a=DC),
                        in_=wq_e[qb])
                    pq = pjp.tile([128, 512], F32, tag="pj")
                    for dc in range(DC):
                        nc.tensor.matmul(pq[:, :],
                                         wq_t[:, dc * 128:(dc + 1) * 128],
                                         xs(dc, 0, 512),
                                         start=(dc == 0), stop=(dc == DC - 1))
                    qcols = slice(qb * S + sg * 512, qb * S + (sg + 1) * 512)
                    norm_rope(pq, qw_t[:, :], QR, qcols, sg, sqp, bcp, swsb,
                              stats, tmps)

        # ---- stage 3+4: attention interleaved with wo projection + RS ----
        # Loop s-quarters (q-groups) outermost: once all 8 heads finish a
        # quarter, its wo projection runs and its ReduceScatter overlaps the
        # next quarter's attention.
        with tc.tile_pool(name="ptp", bufs=6) as ptp, \
             tc.tile_pool(name="p8p", bufs=4) as p8p, \
             tc.tile_pool(name="mgp", bufs=8) as mgp, \
             tc.tile_pool(name="aeps", bufs=4) as aeps, \
             tc.tile_pool(name="otq", bufs=2) as otqp, \
             tc.tile_pool(name="wop", bufs=HQ) as wop, \
             tc.tile_pool(name="oep", bufs=4) as oep, \
             tc.tile_pool(name="ot", bufs=2, space="PSUM") as otp, \
             tc.tile_pool(name="rsp", bufs=2, space="PSUM") as rsp, \
             tc.tile_pool(name="st", bufs=4, space="PSUM") as stp, \
             tc.tile_pool(name="dram", bufs=1, space="DRAM") as dram:
            wos = []
            for c in range(HQ):
                wo_t = wop.tile([128, DG * 512], BF16, tag="wo")
                nc.sync.dma_start(
                    out=wo_t[:, :].rearrange("p (a m) -> p a m", a=DG),
                    in_=wo_e[c])
                wos.append(wo_t)
            # RS chunks: one per sb4 (1.3MB each). A collective's completion
            # spreads cores by ~its transfer time, and the next collective
            # re-aligns to the slowest core -- so small uniform chunks keep
            # both the spread and the post-PE serial tail short.
            # (chunk, qg, sb4 range)
            chunks = [(qg * 4 + s, qg, (s, s + 1))
                      for qg in range(NG) for s in range(4)]
            out_base = [ci * 16 for ci in range(16)]  # chunk base row in out_e
            partials = [dram.tile([(b1 - b0) * 128, D], BF16,
                                  name=f"partial{i}", tag=f"partial{i}")
                        for i, _, (b0, b1) in chunks]
            rs_outs = [dram.tile([(b1 - b0) * 128 // N_CORES, D], BF16,
                                 name=f"rsout{i}", tag=f"rsout{i}")
                       for i, _, (b0, b1) in chunks]

            def epilogue(h, qg, ot, rs, otq):
                rec = aeps.tile([128, 512], F32, tag="arec")
                nc.vector.reciprocal_approx_fast(rec[:, :], rs[:, :])
                nc.vector.tensor_mul(otq[:, h * 512:(h + 1) * 512],
                                     ot[:, :], rec[:, :])

            for qg in range(NG):
                otq = otqp.tile([128, HQ * 512], BF16, tag="otq")
                nkb = (qg + 1) * 4 if causal else SB
                pending = None  # delayed epilogue: keeps PE off the DVE chain
                for h in range(HQ):
                    qbase = h * S + qg * 512
                    ot = otp.tile([128, 512], F32, tag="ot")
                    rs = rsp.tile([128, 512], F32, tag="rs")
                    for kb in range(nkb):
                        # causal: only q >= kb*128 can attend to this k block
                        c0 = max(0, kb * 128 - qg * 512) if causal else 0
                        st = stp.tile([128, 512], F32, tag="st")
                        diag = causal and kb >= qg * 4
                        if diag:
                            # seed PSUM with the causal mask for the diagonal
                            # 128-col block, then accumulate scores on top --
                            # avoids a PE->DVE->ScalarE serial chain per block.
                            nc.tensor.matmul(st[:, c0:c0 + 128], ident[:, :],
                                             maskT[:, kb * 128:(kb + 1) * 128],
                                             start=True, stop=False)
                        nc.tensor.matmul(st[:, c0:],
                                         KR[:, kb * 128:(kb + 1) * 128],
                                         QR[:, qbase + c0:qbase + 512],
                                         start=not diag, stop=True,
                                         skip_group_check=diag)
                        if not causal:
                            mt = mgp.tile([128, 512], F32, tag="mg")
                            nc.sync.dma_start(out=mt[:, :], in_=mask_e[kb, qg])
                            nc.vector.tensor_add(st[:, :], st[:, :], mt[:, :])
                        pt = ptp.tile([128, 512], BF16, tag="pt")
                        nc.scalar.activation(pt[:, c0:], st[:, c0:], Exp)
                        if causal:
                            # fp8 copy of pt into a kb-pair tile; the softmax
                            # denominator is summed with one fp8 DoubleRow
                            # matmul per pair (half the PE cycles of bf16).
                            # e4m3 error on the all-positive rowsum is ~1%
                            # rel; max exp(score) = 197 < 240, no overflow.
                            half = kb % 2
                            if half == 0:
                                p8 = p8p.tile([128, 1024], FP8, tag="p8")
                            if c0 > 0:
                                nc.vector.memset(
                                    p8[:, half * 512:half * 512 + c0], 0.0)
                            nc.vector.tensor_copy(
                                p8[:, half * 512 + c0:(half + 1) * 512],
                                pt[:, c0:])
                            if half == 1:
                                nc.tensor.matmul(
                                    rs[:, :],
                                    ones8[:, :].rearrange(
                                        "p (two m) -> p two m", two=2),
                                    p8[:, :].rearrange(
                                        "p (two n) -> p two n", two=2),
                                    start=(kb == 1), stop=(kb == nkb - 1),
                                    perf_mode=mybir.MatmulPerfMode.DoubleRow,
                                    skip_group_check=True)
                        else:
                            nc.tensor.matmul(rs[:, c0:], ones_f[:, :],
                                             pt[:, c0:],
                                             start=(kb == 0),
                                             stop=(kb == nkb - 1),
                                             skip_group_check=True)
                        nc.tensor.matmul(ot[:, c0:],
                                         Vsd[:, kb * 128:(kb + 1) * 128],
                                         pt[:, c0:],
                                         start=(kb == 0), stop=(kb == nkb - 1),
                                         skip_group_check=True)
                    if pending is not None:
                        epilogue(*pending)
                    pending = (h, qg, ot, rs, otq)
                epilogue(*pending)
                # wo projection for this s-quarter + overlapped RS chunks
                for ci, cqg, (b0, b1) in chunks:
                    if cqg != qg:
                        continue
                    for sb4 in range(b0, b1):
                        for hf in range(2):
                            stg = oep.tile([128, D // 2], BF16, tag="stg")
                            for dg5 in range(5):
                                dg = hf * 5 + dg5
                                po = stp.tile([128, 512], F32, tag="st")
                                for c in range(HQ):
                                    nc.tensor.matmul(
                                        po[:, :],
                                        otq[:, c * 512 + sb4 * 128:
                                            c * 512 + (sb4 + 1) * 128],
                                        wos[c][:, dg * 512:(dg + 1) * 512],
                                        start=(c == 0), stop=(c == HQ - 1))
                                nc.vector.tensor_copy(
                                    stg[:, dg5 * 512:(dg5 + 1) * 512], po[:, :])
                            nc.scalar.dma_start(
                                out=partials[ci][(sb4 - b0) * 128:
                                                 (sb4 - b0 + 1) * 128,
                                                 hf * (D // 2):(hf + 1) * (D // 2)],
                                in_=stg[:, :])
                    nc.gpsimd.collective_compute(
                        "ReduceScatter",
                        mybir.AluOpType.add,
                        replica_groups=[list(range(N_CORES))],
                        ins=[partials[ci].opt()],
                        outs=[rs_outs[ci].opt()],
                    )
                    nrows = (b1 - b0) * 128 // N_CORES
                    nc.sync.dma_start(
                        out=out_e[out_base[ci]:out_base[ci] + nrows, :],
                        in_=rs_outs[ci][:, :])
    nc.compile()
    return nc


def _host_prep(x, wq, wk, wv, wo, q_norm_w, k_norm_w, freqs_cos, freqs_sin,
               mask, causal):
    xs = x[0]                                    # [S, D] f32
    xt = np.ascontiguousarray(xs.T)              # [D, S]
    # p-major swizzle: [sg, p, dc, m] so each load is contiguous per partition
    xt_t = np.ascontiguousarray(
        xt.reshape(DC, 128, NG, 512).transpose(2, 1, 0, 3)).astype(NPBF16)

    idt = np.eye(128, dtype=np.float32).astype(NPBF16)
    p = np.concatenate([np.arange(0, HD, 2), np.arange(1, HD, 2)])
    c64 = np.ascontiguousarray(freqs_cos.T)                   # [64, S]
    s64 = np.ascontiguousarray(freqs_sin.T)
    cosT = np.concatenate([c64, c64], axis=0).astype(NPBF16)  # [128, S]
    sinT = np.concatenate([-s64, s64], axis=0).astype(NPBF16)
    swp = np.zeros((HD, HD), dtype=np.float32)
    swp[np.arange(HD), np.arange(HD) ^ 64] = 1.0
    swp = swp.astype(NPBF16)

    if causal:
        mask_t = np.stack([
            np.ascontiguousarray(mask[b * 128:(b + 1) * 128,
                                      b * 128:(b + 1) * 128].T)
            for b in range(SB)
        ]).astype(NPBF16)
    else:
        mt = np.ascontiguousarray(mask.T)        # [k, q]
        mask_t = np.ascontiguousarray(
            mt.reshape(SB, 128, NG, 512).transpose(0, 2, 1, 3)).astype(np.float32)

    in_maps = []
    for c in range(N_CORES):
        wq_s = wq[c * HQ * HD:(c + 1) * HQ * HD].reshape(HQ, HD, D)[:, p]
        wqT = np.ascontiguousarray(wq_s.reshape(HQ * HD, D).T)   # [D, 1024]
        wq_t = np.ascontiguousarray(
            wqT.reshape(DC, 128, HQ, 128).transpose(2, 1, 0, 3)).astype(NPBF16)
        wkT = np.ascontiguousarray(wk[c * HD:(c + 1) * HD][p].T)  # [D, 128]
        wk_t = np.ascontiguousarray(
            wkT.reshape(DC, 128, 128).transpose(1, 0, 2)).astype(NPBF16)
        wvT = np.ascontiguousarray(wv[c * HD:(c + 1) * HD].T)
        wv_t = np.ascontiguousarray(
            wvT.reshape(DC, 128, 128).transpose(1, 0, 2)).astype(NPBF16)
        woT = np.ascontiguousarray(wo[:, c * HQ * HD:(c + 1) * HQ * HD].T)
        wo_t = np.ascontiguousarray(
            woT.reshape(HQ, 128, DG, 512).transpose(0, 1, 2, 3)).astype(NPBF16)
        qw_v = (q_norm_w[p] / np.sqrt(HD)).astype(np.float32).reshape(HD, 1)
        kw_v = k_norm_w[p].astype(np.float32).reshape(HD, 1)
        in_maps.append({
            "xt": xt_t, "wq": wq_t, "wk": wk_t, "wv": wv_t, "wo": wo_t,
            "cos": cosT, "sin": sinT, "swp": swp, "qw": qw_v, "kw": kw_v,
            "mask": mask_t, "idt": idt,
        })
    return in_maps


def _numpy_fallback(x, wq, wk, wv, wo, q_norm_w, k_norm_w, cache_k, cache_v,
                    freqs_cos, freqs_sin, mask, start_pos):
    bsz, seqlen, _ = x.shape
    xq = (x @ wq.T).reshape(bsz, seqlen, H, HD)
    xk = (x @ wk.T).reshape(bsz, seqlen, KVH, HD)
    xv = (x @ wv.T).reshape(bsz, seqlen, KVH, HD)

    def rms(v, w):
        n = v * (1.0 / np.sqrt((v * v).mean(-1, keepdims=True) + EPS))
        return n * w

    def rope(v):
        vr = v.reshape(*v.shape[:-1], HD // 2, 2)
        ve, vo = vr[..., 0], vr[..., 1]
        c = freqs_cos[None, :, None, :]
        s = freqs_sin[None, :, None, :]
        oe = ve * c - vo * s
        oo = ve * s + vo * c
        return np.stack([oe, oo], axis=-1).reshape(v.shape)

    xq = rope(rms(xq, q_norm_w))
    xk = rope(rms(xk, k_norm_w))
    ck = np.array(cache_k)
    cv = np.array(cache_v)
    ck[:bsz, start_pos:start_pos + seqlen] = xk
    cv[:bsz, start_pos:start_pos + seqlen] = xv
    kv_len = start_pos + seqlen
    keys = np.repeat(ck[:bsz, :kv_len], H // KVH, axis=2)
    values = np.repeat(cv[:bsz, :kv_len], H // KVH, axis=2)
    sc = np.einsum('bqhd,bkhd->bhqk', xq, keys) / np.sqrt(HD)
    if mask is not None:
        sc = sc + mask[None, None, :, :]
    sc = sc - sc.max(-1, keepdims=True)
    e = np.exp(sc)
    probs = e / e.sum(-1, keepdims=True)
    out = np.einsum('bhqk,bkhd->bqhd', probs, values)
    return (out.reshape(bsz, seqlen, H * HD) @ wo.T).astype(np.float32)


def _run(trace=False, **inputs):
    x = np.asarray(inputs["x"], dtype=np.float32)
    wq = np.asarray(inputs["wq"], dtype=np.float32)
    wk = np.asarray(inputs["wk"], dtype=np.float32)
    wv = np.asarray(inputs["wv"], dtype=np.float32)
    wo = np.asarray(inputs["wo"], dtype=np.float32)
    q_norm_w = np.asarray(inputs["q_norm_w"], dtype=np.float32)
    k_norm_w = np.asarray(inputs["k_norm_w"], dtype=np.float32)
    freqs_cos = np.asarray(inputs["freqs_cos"], dtype=np.float32)
    freqs_sin = np.asarray(inputs["freqs_sin"], dtype=np.float32)
    mask = np.asarray(inputs["mask"], dtype=np.float32)
    start_pos = int(inputs.get("start_pos", 0))

    if start_pos != 0 or x.shape != (1, S, D):
        return _numpy_fallback(
            x, wq, wk, wv, wo, q_norm_w, k_norm_w,
            np.asarray(inputs["cache_k"]), np.asarray(inputs["cache_v"]),
            freqs_cos, freqs_sin, mask, start_pos), None

    causal = bool(
        (mask == np.triu(np.full((S, S), -1e9, dtype=np.float32), k=1)).all())

    key = ("nc", causal)
    if key not in _cache:
        _cache[key] = _build(causal)
    nc = _cache[key]
    in_maps = _host_prep(x, wq, wk, wv, wo, q_norm_w, k_norm_w,
                         freqs_cos, freqs_sin, mask, causal)
    import os as _os
    _tc = list(range(N_CORES)) if _os.environ.get("TRACE_ALL") else None
    res = None
    for _attempt in range(3):
        try:
            res = run_bass_kernel_spmd(nc, in_maps,
                                       core_ids=list(range(N_CORES)),
                                       trace=trace, trace_cores=_tc)
            break
        except Exception:
            # transient device errors (e.g. a wedged core from an earlier
            # run) usually clear on retry
            if _attempt == 2:
                raise
    out = np.empty((S, D), dtype=np.float32)
    chunks = [(ci, ci * 128, ci * 16, 16) for ci in range(16)]
    for r in range(N_CORES):
        o = np.asarray(res.results[r]["out"], dtype=np.float32)
        if RS_CHUNKED:
            for ci, gbase, obase, rows in chunks:
                out[gbase + r * rows:gbase + (r + 1) * rows] = \
                    o[obase:obase + rows]
        else:
            out[r * 256:(r + 1) * 256] = o
    return out.reshape(1, S, D), res


def kernel(**inputs) -> np.ndarray:
    out, _ = _run(trace=False, **inputs)
    return out



# revision 62
# speedup vs baseline: 1.0114x; 1.0114x over previous
# Distributed Trainium2 kernel for the GQA attention block
# (nn_Attention_52621939311076).
#
# Sharding: tensor-parallel over heads across 8 NeuronCores. Core c owns
# q-heads [8c, 8c+8) and kv-head c (GQA group stays local). x is replicated,
# wq/wk/wv are sharded on the output dim, wo on the input dim; partial wo
# outputs are summed with an on-device ReduceScatter and the rank slices are
# concatenated on the host.
#
# Everything on device lives in a transposed [feature, seq] layout so that no
# on-chip transposes are needed anywhere:
#   - projections produce Q^T/K^T (head_dim on partitions) and V in [s, d],
#   - RMSNorm reduction over head_dim uses a ones-matmul (partition reduce),
#   - RoPE pairs are (even, odd) partition halves via a host-side permutation
#     of the wq/wk output dims,
#   - attention computes S^T = K^T.T-stationary @ Q^T, softmax row sums via a
#     ones-matmul, O^T = V-stationary @ P^T,
#   - the wo matmul consumes O^T directly.
# Matmuls run in bf16 (4x the fp32 TensorE rate), accumulating in fp32 PSUM.
import numpy as np
import ml_dtypes

import concourse.bass as bass
import concourse.bacc as bacc
import concourse.mybir as mybir
import concourse.tile as tile
from concourse import bass_isa
from concourse.bass_utils import run_bass_kernel_spmd

# enable the neuronxcc ldweights optimization (measured ~12us faster)
try:
    from concourse import compiler_utils as _cu
    _fl = _cu.get_compiler_flags()
    _cu.set_compiler_flags([f.replace("--enable-ldw-opt=false",
                                      "--enable-ldw-opt=true") for f in _fl])
except Exception:
    pass

BF16 = mybir.dt.bfloat16
F32 = mybir.dt.float32
FP8 = mybir.dt.float8e4
NPBF16 = ml_dtypes.bfloat16

N_CORES = 8
S = 2048          # sequence length
D = 5120          # model dim
H = 64            # q heads (global)
KVH = 8           # kv heads (global)
HD = 128          # head dim
HQ = H // N_CORES  # q heads per core
DC = D // 128     # contraction chunks for the projections
SB = S // 128     # 128-row seq blocks
NG = S // 512     # 512-col seq groups
DG = D // 512     # 512-col output groups for wo
EPS = 1e-6

_cache = {}
RS_CHUNKED = True



def _build(causal: bool):
    nc = bacc.Bacc("TRN2", target_bir_lowering=False, debug=False,
                   num_devices=N_CORES)

    xt_e = nc.dram_tensor("xt", [NG, 128, DC, 512], BF16, kind="ExternalInput")
    wq_e = nc.dram_tensor("wq", [HQ, 128, DC, 128], BF16, kind="ExternalInput")
    wk_e = nc.dram_tensor("wk", [128, DC, 128], BF16, kind="ExternalInput")
    wv_e = nc.dram_tensor("wv", [128, DC, 128], BF16, kind="ExternalInput")
    wo_e = nc.dram_tensor("wo", [HQ, 128, DG, 512], BF16, kind="ExternalInput")
    cos_e = nc.dram_tensor("cos", [128, S], BF16, kind="ExternalInput")
    sin_e = nc.dram_tensor("sin", [128, S], BF16, kind="ExternalInput")
    swp_e = nc.dram_tensor("swp", [128, 128], BF16, kind="ExternalInput")
    qw_e = nc.dram_tensor("qw", [128, 1], F32, kind="ExternalInput")
    kw_e = nc.dram_tensor("kw", [128, 1], F32, kind="ExternalInput")
    idt_e = nc.dram_tensor("idt", [128, 128], BF16, kind="ExternalInput")
    if causal:
        mask_e = nc.dram_tensor("mask", [SB, 128, 128], BF16, kind="ExternalInput")
    else:
        mask_e = nc.dram_tensor("mask", [SB, NG, 128, 512], F32,
                                kind="ExternalInput")
    out_e = nc.dram_tensor("out", [S // N_CORES, D], BF16, kind="ExternalOutput")

    mult = mybir.AluOpType.mult
    Exp = mybir.ActivationFunctionType.Exp
    Sqrt = mybir.ActivationFunctionType.Sqrt
    Square = mybir.ActivationFunctionType.Square

    with tile.TileContext(nc) as tc, \
         tc.tile_pool(name="persist", bufs=1) as persist:
        def single(shape, dtype, name):
            return persist.tile(shape, dtype, name=name, tag=name)

        # ---- persistent SBUF tensors -------------------------------------
        QR = single([128, HQ * S], BF16, "QR")     # roped q, [d, s] per head
        KR = single([128, S], BF16, "KR")          # roped k, [d, s]
        Vsd = single([128, S], BF16, "Vsd")        # v in [s, d], s-block b at cols b*128
        cosT = single([128, S], BF16, "cosT")   # cos duplicated on both halves
        sinT = single([128, S], BF16, "sinT")   # [-sin; +sin]
        swp_t = single([128, 128], BF16, "swp_t")
        qw_t = single([128, 1], F32, "qw_t")
        kw_t = single([128, 1], F32, "kw_t")
        ones_f = single([128, 128], BF16, "ones_f")  # full ones: bcast rowsum
        ones8 = single([128, 256], FP8, "ones8")  # fp8 pair-ones: DR rowsum
        eps_t = single([128, 1], F32, "eps_t")
        ident = single([128, 128], BF16, "ident")
        if causal:
            maskT = single([128, SB * 128], BF16, "maskT")

        nc.gpsimd.dma_start(out=cosT[:, :], in_=cos_e[:, :])
        nc.gpsimd.dma_start(out=sinT[:, :], in_=sin_e[:, :])
        nc.gpsimd.dma_start(out=swp_t[:, :], in_=swp_e[:, :])
        nc.gpsimd.dma_start(out=qw_t[:, :], in_=qw_e[:, :])
        nc.gpsimd.dma_start(out=kw_t[:, :], in_=kw_e[:, :])
        nc.vector.memset(ones_f[:, :], 1.0)
        nc.vector.memset(ones8[:, :], 1.0)
        nc.vector.memset(eps_t[:, :], EPS)
        nc.gpsimd.dma_start(out=ident[:, :], in_=idt_e[:, :])
        if causal:
            for b in range(SB):
                nc.gpsimd.dma_start(out=maskT[:, b * 128:(b + 1) * 128],
                                  in_=mask_e[b])

        # ---- stage 1+2: projections + rmsnorm + rope ---------------------
        def norm_rope(pj, w_ap, dst, dst_cols, sg, sqp, bcp, swsb, stats,
                      tmps):
            """pj: PSUM [128,512] projection block; writes roped dst[:, dst_cols]."""
            sq = sqp.tile([128, 512], BF16, tag="sq")
            nc.scalar.activation(sq[:, :], pj[:, :], Square)
            # partition-sum of sq broadcast to all partitions, on GpSimd (idle
            # engine) instead of a ones-matmul on the PE
            bc = bcp.tile([128, 512], F32, tag="bc")
            nc.gpsimd.partition_all_reduce(bc[:, :], sq[:, :], 128,
                                           bass_isa.ReduceOp.add)
            rstd = stats.tile([128, 512], F32, tag="rstd")
            nc.scalar.activation(rstd[:, :], bc[:, :], Sqrt, bias=eps_t[:, :],
                                 scale=1.0 / HD)
            rec = stats.tile([128, 512], F32, tag="rec")
            nc.vector.reciprocal_approx_fast(rec[:, :], rstd[:, :])
            qn = stats.tile([128, 512], BF16, tag="qn")
            # qn = (pj * w) * rec  -- normalized, weighted, cast to bf16
            nc.vector.scalar_tensor_tensor(qn[:, :], pj[:, :], w_ap, rec[:, :],
                                           op0=mult, op1=mult)
            # rope: out = qn*cos2 + swap_halves(qn)*[-sin; sin]; the partition
            # half-swap runs on the DMA engines, not the PE.
            cs = cosT[:, sg * 512:(sg + 1) * 512]
            sn = sinT[:, sg * 512:(sg + 1) * 512]
            sw = swsb.tile([128, 512], BF16, tag="sw")
            nc.sync.dma_start(out=sw[0:64, :], in_=qn[64:128, :])
            nc.sync.dma_start(out=sw[64:128, :], in_=qn[0:64, :])
            t1 = tmps.tile([128, 512], BF16, tag="t1")
            t2 = tmps.tile([128, 512], BF16, tag="t2")
            nc.vector.tensor_mul(t1[:, :], qn[:, :], cs)
            nc.vector.tensor_mul(t2[:, :], sw[:, :], sn)
            nc.vector.tensor_add(dst[:, dst_cols], t1[:, :], t2[:, :])

        with tc.tile_pool(name="xp", bufs=3) as xp, \
             tc.tile_pool(name="wqp", bufs=2) as wqp, \
             tc.tile_pool(name="wkvp", bufs=2) as wkvp, \
             tc.tile_pool(name="sqp", bufs=2) as sqp, \
             tc.tile_pool(name="stats", bufs=3) as stats, \
             tc.tile_pool(name="tmps", bufs=4) as tmps, \
             tc.tile_pool(name="vtp", bufs=2) as vtp, \
             tc.tile_pool(name="swsb", bufs=3) as swsb, \
             tc.tile_pool(name="bcp", bufs=3) as bcp, \
             tc.tile_pool(name="pj", bufs=3, space="PSUM") as pjp, \
             tc.tile_pool(name="pv", bufs=2, space="PSUM") as pvp:
            for sg in range(NG):
                # x^T for this s-group: two half tiles [128, 20*512]
                xhs = []
                for hf in range(2):
                    xh = xp.tile([128, 20 * 512], BF16, tag="xh")
                    nc.sync.dma_start(
                        out=xh[:, :].rearrange("p (a m) -> p a m", a=20),
                        in_=xt_e[sg, :, hf * 20:(hf + 1) * 20, :])
                    xhs.append(xh)

                def xs(dc, c0, w):
                    return xhs[dc // 20][:, (dc % 20) * 512 + c0:
                                         (dc % 20) * 512 + c0 + w]

                # K/V weights for this s-group (reloaded each sg; cheap)
                wk_t = wkvp.tile([128, DC * 128], BF16, tag="wkv")
                nc.sync.dma_start(
                    out=wk_t[:, :].rearrange("p (a m) -> p a m", a=DC),
                    in_=wk_e[:, :, :])
                wv_t = wkvp.tile([128, DC * 128], BF16, tag="wkv")
                nc.sync.dma_start(
                    out=wv_t[:, :].rearrange("p (a m) -> p a m", a=DC),
                    in_=wv_e[:, :, :])
                cols = slice(sg * 512, (sg + 1) * 512)
                pk = pjp.tile([128, 512], F32, tag="pj")
                for dc in range(DC):
                    nc.tensor.matmul(pk[:, :],
                                     wk_t[:, dc * 128:(dc + 1) * 128],
                                     xs(dc, 0, 512),
                                     start=(dc == 0), stop=(dc == DC - 1))
                norm_rope(pk, kw_t[:, :], KR, cols, sg, sqp, bcp, swsb,
                          stats, tmps)
                # V^T [d, s] with 512-wide matmuls (like K), then PE-transpose
                # each 128x128 block into the [s, d] layout the AV matmul needs.
                pvt = pvp.tile([128, 512], F32, tag="pv")
                for dc in range(DC):
                    nc.tensor.matmul(pvt[:, :],
                                     wv_t[:, dc * 128:(dc + 1) * 128],
                                     xs(dc, 0, 512),
                                     start=(dc == 0), stop=(dc == DC - 1))
                vt_sb = vtp.tile([128, 512], BF16, tag="vt")
                nc.vector.tensor_copy(vt_sb[:, :], pvt[:, :])
                for sb4 in range(4):
                    sb = sg * 4 + sb4
                    # DMA crossbar transpose straight into the [s, d] layout
                    nc.sync.dma_start_transpose(
                        out=Vsd[:, sb * 128:(sb + 1) * 128],
                        in_=vt_sb[:, sb4 * 128:(sb4 + 1) * 128])
                # Q heads
                for qb in range(HQ):
                    wq_t = wqp.tile([128, DC * 128], BF16, tag="wq")
                    nc.sync.dma_start(
                        out=wq_t[:, :].rearrange("p (a m) -> p a m", a=DC),
                        in_=wq_e[qb])
                    pq = pjp.tile([128, 512], F32, tag="pj")
                    for dc in range(DC):
                        nc.tensor.matmul(pq[:, :],
                                         wq_t[:, dc * 128:(dc + 1) * 128],
                                         xs(dc, 0, 512),
                                         start=(dc == 0), stop=(dc == DC - 1))
                    qcols = slice(qb * S + sg * 512, qb * S + (sg + 1) * 512)
                    norm_rope(pq, qw_t[:, :], QR, qcols, sg, sqp, bcp, swsb,
                              stats, tmps)

        # ---- stage 3+4: attention interleaved with wo projection + RS ----
        # Loop s-quarters (q-groups) outermost: once all 8 heads finish a
        # quarter, its wo projection runs and its ReduceScatter overlaps the
        # next quarter's attention.
        with tc.tile_pool(name="ptp", bufs=6) as ptp, \
             tc.tile_pool(name="p8p", bufs=4) as p8p, \
             tc.tile_pool(name="mgp", bufs=8) as mgp, \
             tc.tile_pool(name="aeps", bufs=4) as aeps, \
             tc.tile_pool(name="otq", bufs=2) as otqp, \
             tc.tile_pool(name="wop", bufs=HQ) as wop, \
             tc.tile_pool(name="oep", bufs=4) as oep, \
             tc.tile_pool(name="ot", bufs=2, space="PSUM") as otp, \
             tc.tile_pool(name="rsp", bufs=2, space="PSUM") as rsp, \
             tc.tile_pool(name="st", bufs=4, space="PSUM") as stp, \
             tc.tile_pool(name="dram", bufs=1, space="DRAM") as dram:
            wos = []
            for c in range(HQ):
                wo_t = wop.tile([128, DG * 512], BF16, tag="wo")
                nc.sync.dma_start(
                    out=wo_t[:, :].rearrange("p (a m) -> p a m", a=DG),
                    in_=wo_e[c])
                wos.append(wo_t)
            # RS chunks: one per sb4 (1.3MB each). A collective's completion
            # spreads cores by ~its transfer time, and the next collective
            # re-aligns to the slowest core -- so small uniform chunks keep
            # both the spread and the post-PE serial tail short.
            # (chunk, qg, sb4 range)
            chunks = [(qg * 4 + s, qg, (s, s + 1))
                      for qg in range(NG) for s in range(4)]
            out_base = [ci * 16 for ci in range(16)]  # chunk base row in out_e
            partials = [dram.tile([(b1 - b0) * 128, D], BF16,
                                  name=f"partial{i}", tag=f"partial{i}")
                        for i, _, (b0, b1) in chunks]
            rs_outs = [dram.tile([(b1 - b0) * 128 // N_CORES, D], BF16,
                                 name=f"rsout{i}", tag=f"rsout{i}")
                       for i, _, (b0, b1) in chunks]

            def epilogue(h, qg, ot, rs, otq):
                rec = aeps.tile([128, 512], F32, tag="arec")
                nc.vector.reciprocal_approx_fast(rec[:, :], rs[:, :])
                nc.vector.tensor_mul(otq[:, h * 512:(h + 1) * 512],
                                     ot[:, :], rec[:, :])

            for qg in range(NG):
                otq = otqp.tile([128, HQ * 512], BF16, tag="otq")
                nkb = (qg + 1) * 4 if causal else SB
                pending = None  # delayed epilogue: keeps PE off the DVE chain
                for h in range(HQ):
                    qbase = h * S + qg * 512
                    ot = otp.tile([128, 512], F32, tag="ot")
                    rs = rsp.tile([128, 512], F32, tag="rs")
                    for kb in range(nkb):
                        # causal: only q >= kb*128 can attend to this k block
                        c0 = max(0, kb * 128 - qg * 512) if causal else 0
                        st = stp.tile([128, 512], F32, tag="st")
                        diag = causal and kb >= qg * 4
                        if diag:
                            # seed PSUM with the causal mask for the diagonal
                            # 128-col block, then accumulate scores on top --
                            # avoids a PE->DVE->ScalarE serial chain per block.
                            nc.tensor.matmul(st[:, c0:c0 + 128], ident[:, :],
                                             maskT[:, kb * 128:(kb + 1) * 128],
                                             start=True, stop=False)
                        nc.tensor.matmul(st[:, c0:],
                                         KR[:, kb * 128:(kb + 1) * 128],
                                         QR[:, qbase + c0:qbase + 512],
                                         start=not diag, stop=True,
                                         skip_group_check=diag)
                        if not causal:
                            mt = mgp.tile([128, 512], F32, tag="mg")
                            nc.sync.dma_start(out=mt[:, :], in_=mask_e[kb, qg])
                            nc.vector.tensor_add(st[:, :], st[:, :], mt[:, :])
                        pt = ptp.tile([128, 512], BF16, tag="pt")
                        nc.scalar.activation(pt[:, c0:], st[:, c0:], Exp)
                        if causal:
                            # fp8 copy of pt into a kb-pair tile; the softmax
                            # denominator is summed with one fp8 DoubleRow
                            # matmul per pair (half the PE cycles of bf16).
                            # e4m3 error on the all-positive rowsum is ~1%
                            # rel; max exp(score) = 197 < 240, no overflow.
                            half = kb % 2
                            if half == 0:
                                p8 = p8p.tile([128, 1024], FP8, tag="p8")
                            if c0 > 0:
                                nc.vector.memset(
                                    p8[:, half * 512:half * 512 + c0], 0.0)
                            nc.vector.tensor_copy(
                                p8[:, half * 512 + c0:(half + 1) * 512],
                                pt[:, c0:])
                            if half == 1:
                                nc.tensor.matmul(
                                    rs[:, :],
                                    ones8[:, :].rearrange(
                                        "p (two m) -> p two m", two=2),
                                    p8[:, :].rearrange(
                                        "p (two n) -> p two n", two=2),
                                    start=(kb == 1), stop=(kb == nkb - 1),
                                    perf_mode=mybir.MatmulPerfMode.DoubleRow,
                                    skip_group_check=True)
                        else:
                            nc.tensor.matmul(rs[:, c0:], ones_f[:, :],
                                             pt[:, c0:],
                                             start=(kb == 0),
                                             stop=(kb == nkb - 1),
                                             skip_group_check=True)
                        nc.tensor.matmul(ot[:, c0:],
                                         Vsd[:, kb * 128:(kb + 1) * 128],
                                         pt[:, c0:],
                                         start=(kb == 0), stop=(kb == nkb - 1),
                                         skip_group_check=True)
                    if pending is not None:
                        epilogue(*pending)
                    pending = (h, qg, ot, rs, otq)
                epilogue(*pending)
                # wo projection for this s-quarter + overlapped RS chunks
                for ci, cqg, (b0, b1) in chunks:
                    if cqg != qg:
                        continue
                    for sb4 in range(b0, b1):
                        for hf in range(2):
                            stg = oep.tile([128, D // 2], BF16, tag="stg")
                            for dg5 in range(5):
                                dg = hf * 5 + dg5
                                po = stp.tile([128, 512], F32, tag="st")
                                for c in range(HQ):
                                    nc.tensor.matmul(
                                        po[:, :],
                                        otq[:, c * 512 + sb4 * 128:
                                            c * 512 + (sb4 + 1) * 128],
                                        wos[c][:, dg * 512:(dg + 1) * 512],
                                        start=(c == 0), stop=(c == HQ - 1))
                                nc.vector.tensor_copy(
                                    stg[:, dg5 * 512:(dg5 + 1) * 512], po[:, :])
                            nc.scalar.dma_start(
                                out=partials[ci][(sb4 - b0) * 128:
                                                 (sb4 - b0 + 1) * 128,
                                                 hf * (D // 2):(hf + 1) * (D // 2)],
                                in_=stg[:, :])
                    nc.gpsimd.collective_compute(
                        "ReduceScatter",
                        mybir.AluOpType.add,
                        replica_groups=[list(range(N_CORES))],
                        ins=[partials[ci].opt()],
                        outs=[rs_outs[ci].opt()],
                    )
                    nrows = (b1 - b0) * 128 // N_CORES
                    nc.sync.dma_start(
                        out=out_e[out_base[ci]:out_base[ci] + nrows, :],
                        in_=rs_outs[ci][:, :])
    nc.compile()
    return nc


def _host_prep(x, wq, wk, wv, wo, q_norm_w, k_norm_w, freqs_cos, freqs_sin,
               mask, causal):
    xs = x[0]                                    # [S, D] f32
    xt = np.ascontiguousarray(xs.T)              # [D, S]
    # p-major swizzle: [sg, p, dc, m] so each load is contiguous per partition
    xt_t = np.ascontiguousarray(
        xt.reshape(DC, 128, NG, 512).transpose(2, 1, 0, 3)).astype(NPBF16)

    idt = np.eye(128, dtype=np.float32).astype(NPBF16)
    p = np.concatenate([np.arange(0, HD, 2), np.arange(1, HD, 2)])
    c64 = np.ascontiguousarray(freqs_cos.T)                   # [64, S]
    s64 = np.ascontiguousarray(freqs_sin.T)
    cosT = np.concatenate([c64, c64], axis=0).astype(NPBF16)  # [128, S]
    sinT = np.concatenate([-s64, s64], axis=0).astype(NPBF16)
    swp = np.zeros((HD, HD), dtype=np.float32)
    swp[np.arange(HD), np.arange(HD) ^ 64] = 1.0
    swp = swp.astype(NPBF16)

    if causal:
        mask_t = np.stack([
            np.ascontiguousarray(mask[b * 128:(b + 1) * 128,
                                      b * 128:(b + 1) * 128].T)
            for b in range(SB)
        ]).astype(NPBF16)
    else:
        mt = np.ascontiguousarray(mask.T)        # [k, q]
        mask_t = np.ascontiguousarray(
            mt.reshape(SB, 128, NG, 512).transpose(0, 2, 1, 3)).astype(np.float32)

    in_maps = []
    for c in range(N_CORES):
        wq_s = wq[c * HQ * HD:(c + 1) * HQ * HD].reshape(HQ, HD, D)[:, p]
        wqT = np.ascontiguousarray(wq_s.reshape(HQ * HD, D).T)   # [D, 1024]
        wq_t = np.ascontiguousarray(
            wqT.reshape(DC, 128, HQ, 128).transpose(2, 1, 0, 3)).astype(NPBF16)
        wkT = np.ascontiguousarray(wk[c * HD:(c + 1) * HD][p].T)  # [D, 128]
        wk_t = np.ascontiguousarray(
            wkT.reshape(DC, 128, 128).transpose(1, 0, 2)).astype(NPBF16)
        wvT = np.ascontiguousarray(wv[c * HD:(c + 1) * HD].T)
        wv_t = np.ascontiguousarray(
            wvT.reshape(DC, 128, 128).transpose(1, 0, 2)).astype(NPBF16)
        woT = np.ascontiguousarray(wo[:, c * HQ * HD:(c + 1) * HQ * HD].T)
        wo_t = np.ascontiguousarray(
            woT.reshape(HQ, 128, DG, 512).transpose(0, 1, 2, 3)).astype(NPBF16)
        qw_v = (q_norm_w[p] / np.sqrt(HD)).astype(np.float32).reshape(HD, 1)
        kw_v = k_norm_w[p].astype(np.float32).reshape(HD, 1)
        in_maps.append({
            "xt": xt_t, "wq": wq_t, "wk": wk_t, "wv": wv_t, "wo": wo_t,
            "cos": cosT, "sin": sinT, "swp": swp, "qw": qw_v, "kw": kw_v,
            "mask": mask_t, "idt": idt,
        })
    return in_maps


def _numpy_fallback(x, wq, wk, wv, wo, q_norm_w, k_norm_w, cache_k, cache_v,
                    freqs_cos, freqs_sin, mask, start_pos):
    bsz, seqlen, _ = x.shape
    xq = (x @ wq.T).reshape(bsz, seqlen, H, HD)
    xk = (x @ wk.T).reshape(bsz, seqlen, KVH, HD)
    xv = (x @ wv.T).reshape(bsz, seqlen, KVH, HD)

    def rms(v, w):
        n = v * (1.0 / np.sqrt((v * v).mean(-1, keepdims=True) + EPS))
        return n * w

    def rope(v):
        vr = v.reshape(*v.shape[:-1], HD // 2, 2)
        ve, vo = vr[..., 0], vr[..., 1]
        c = freqs_cos[None, :, None, :]
        s = freqs_sin[None, :, None, :]
        oe = ve * c - vo * s
        oo = ve * s + vo * c
        return np.stack([oe, oo], axis=-1).reshape(v.shape)

    xq = rope(rms(xq, q_norm_w))
    xk = rope(rms(xk, k_norm_w))
    ck = np.array(cache_k)
    cv = np.array(cache_v)
    ck[:bsz, start_pos:start_pos + seqlen] = xk
    cv[:bsz, start_pos:start_pos + seqlen] = xv
    kv_len = start_pos + seqlen
    keys = np.repeat(ck[:bsz, :kv_len], H // KVH, axis=2)
    values = np.repeat(cv[:bsz, :kv_len], H // KVH, axis=2)
    sc = np.einsum('bqhd,bkhd->bhqk', xq, keys) / np.sqrt(HD)
    if mask is not None:
        sc = sc + mask[None, None, :, :]
    sc = sc - sc.max(-1, keepdims=True)
    e = np.exp(sc)
    probs = e / e.sum(-1, keepdims=True)
    out = np.einsum('bhqk,bkhd->bqhd', probs, values)
    return (out.reshape(bsz, seqlen, H * HD) @ wo.T).astype(np.float32)


def _run(trace=False, **inputs):
    x = np.asarray(inputs["x"], dtype=np.float32)
    wq = np.asarray(inputs["wq"], dtype=np.float32)
    wk = np.asarray(inputs["wk"], dtype=np.float32)
    wv = np.asarray(inputs["wv"], dtype=np.float32)
    wo = np.asarray(inputs["wo"], dtype=np.float32)
    q_norm_w = np.asarray(inputs["q_norm_w"], dtype=np.float32)
    k_norm_w = np.asarray(inputs["k_norm_w"], dtype=np.float32)
    freqs_cos = np.asarray(inputs["freqs_cos"], dtype=np.float32)
    freqs_sin = np.asarray(inputs["freqs_sin"], dtype=np.float32)
    mask = np.asarray(inputs["mask"], dtype=np.float32)
    start_pos = int(inputs.get("start_pos", 0))

    if start_pos != 0 or x.shape != (1, S, D):
        return _numpy_fallback(
            x, wq, wk, wv, wo, q_norm_w, k_norm_w,
            np.asarray(inputs["cache_k"]), np.asarray(inputs["cache_v"]),
            freqs_cos, freqs_sin, mask, start_pos), None

    causal = bool(
        (mask == np.triu(np.full((S, S), -1e9, dtype=np.float32), k=1)).all())

    key = ("nc", causal)
    if key not in _cache:
        _cache[key] = _build(causal)
    nc = _cache[key]
    in_maps = _host_prep(x, wq, wk, wv, wo, q_norm_w, k_norm_w,
                         freqs_cos, freqs_sin, mask, causal)
    import os as _os
    _tc = list(range(N_CORES)) if _os.environ.get("TRACE_ALL") else None
    res = None
    for _attempt in range(3):
        try:
            res = run_bass_kernel_spmd(nc, in_maps,
                                       core_ids=list(range(N_CORES)),
                                       trace=trace, trace_cores=_tc)
            break
        except Exception:
            # transient device errors (e.g. a wedged core from an earlier
            # run) usually clear on retry
            if _attempt == 2:
                raise
    out = np.empty((S, D), dtype=np.float32)
    chunks = [(ci, ci * 128, ci * 16, 16) for ci in range(16)]
    for r in range(N_CORES):
        o = np.asarray(res.results[r]["out"], dtype=np.float32)
        if RS_CHUNKED:
            for ci, gbase, obase, rows in chunks:
                out[gbase + r * rows:gbase + (r + 1) * rows] = \
                    o[obase:obase + rows]
        else:
            out[r * 256:(r + 1) * 256] = o
    return out.reshape(1, S, D), res


def kernel(**inputs) -> np.ndarray:
    out, _ = _run(trace=False, **inputs)
    return out

